# revision 19
# baseline (speedup 1.0000x reference)
"""Trainium2 Bass kernel for a GPT-2 style transformer block.

Problem: nn_Block (B=4, T=2048, C=768, H=12 heads, causal attention, MLP 4x).
Sharding over 8 NeuronCores: core c -> batch b=c//2, head-half hf=c%2
(6 heads each). MLP/proj are token-split (each core handles 1024 tokens of
its batch); one pair AllGather exchanges attention outputs.

Layout: activations are feature-major ([C, T], features on partitions) so
weights serve directly as matmul lhsT. LayerNorm stats (per-token, across
partitions) are computed with ones-vector matmuls on the TensorEngine and
broadcast back with K=1 matmuls. Softmax denominators come free from an
appended ones-column on V during the attn@V matmul.
"""
import math
import contextlib
import numpy as np
import ml_dtypes

import concourse.bass as bass
import concourse.bacc as bacc
import concourse.tile as tile
from concourse import mybir
from concourse.bass_utils import run_bass_kernel_spmd

F32 = mybir.dt.float32
BF16 = mybir.dt.bfloat16
AF = mybir.ActivationFunctionType

B, T, C = 4, 2048, 768
H, D = 12, 64
NCORES = 8
HPC = H // 2          # heads per core = 6
CC = C // 128         # feature tiles = 6
TT = T // 128         # token tiles = 16
TH = T // 2           # tokens per core for MLP/proj = 1024
FF = 4 * C            # mlp hidden = 3072
QC = 512              # q-chunk width for attention
NQC = T // QC         # 4
EPS = 1e-5


def build_program(debug_taps=False):
    nc = bacc.Bacc(None, target_bir_lowering=False)

    # ---- per-core external inputs ----
    xT_d = nc.dram_tensor("xT", [C, T], F32, kind="ExternalInput")
    xTh_d = nc.dram_tensor("xTh", [C, TH], F32, kind="ExternalInput")
    wqk_d = nc.dram_tensor("wqk", [C, 2 * HPC * D], F32, kind="ExternalInput")
    bqk_d = nc.dram_tensor("bqk", [2 * HPC * D], F32, kind="ExternalInput")
    wv_d = nc.dram_tensor("wv", [C, HPC * D], F32, kind="ExternalInput")
    bv_d = nc.dram_tensor("bv", [1, HPC * D], F32, kind="ExternalInput")
    wproj_d = nc.dram_tensor("wproj", [C, C], F32, kind="ExternalInput")
    bproj_d = nc.dram_tensor("bproj", [C], F32, kind="ExternalInput")
    wfc_d = nc.dram_tensor("wfc", [C, FF], F32, kind="ExternalInput")
    bfc_d = nc.dram_tensor("bfc", [FF], F32, kind="ExternalInput")
    wfc2_d = nc.dram_tensor("wfc2", [FF, C], BF16, kind="ExternalInput")
    bfc2_d = nc.dram_tensor("bfc2", [C], F32, kind="ExternalInput")

    outT_d = nc.dram_tensor("outT", [C, TH], F32, kind="ExternalOutput")

    taps = {}
    if debug_taps:
        taps["xln"] = nc.dram_tensor("tap_xln", [C, T], F32, kind="ExternalOutput")
        taps["qkT"] = nc.dram_tensor("tap_qkT", [C, T], F32, kind="ExternalOutput")
        taps["v"] = nc.dram_tensor("tap_v", [T, HPC * D], F32, kind="ExternalOutput")
        taps["yT"] = nc.dram_tensor("tap_yT", [HPC * D, T], F32, kind="ExternalOutput")
        taps["x2"] = nc.dram_tensor("tap_x2", [C, TH], F32, kind="ExternalOutput")
        taps["xln2"] = nc.dram_tensor("tap_xln2", [C, TH], F32, kind="ExternalOutput")

    # ---- inline constants ----
    inv768 = nc.inline_tensor(np.full((128, 1), 1.0 / C, np.float32), name="inv768")
    ones_1x128 = nc.inline_tensor(np.ones((1, 128), np.float32), name="ones_1x128")
    negones_1x128 = nc.inline_tensor(-np.ones((1, 128), np.float32), name="negones_1x128")
    ones_1x64 = nc.inline_tensor(np.ones((1, 64), np.float32), name="ones_1x64")
    tri = np.triu(np.ones((128, 128), np.float32))  # tri[k,q]=1 iff k<=q
    tri_mask = nc.inline_tensor(tri, name="tri_mask")

    def rearr_in(d):  # DRAM [(c p), t] -> AP [p, c, t]
        return d[:].rearrange("(c p) t -> p c t", p=128)

    def rearr_vec(d):  # DRAM [(c p)] -> AP [p, c]
        return d[:].rearrange("(c p) -> p c", p=128)

    FT = FF // 128  # 24

    with tile.TileContext(nc) as tc:
        with tc.tile_pool(name="consts", bufs=1) as consts:
            inv768_sb = consts.tile([128, 1], F32)
            nc.sync.dma_start(inv768_sb[:], inv768[:])
            ones128_sb = consts.tile([1, 128], F32)
            nc.sync.dma_start(ones128_sb[:], ones_1x128[:])
            negones128_sb = consts.tile([1, 128], F32)
            nc.sync.dma_start(negones128_sb[:], negones_1x128[:])
            ones64_sb = consts.tile([1, 64], F32)
            nc.sync.dma_start(ones64_sb[:], ones_1x64[:])
            tri_sb = consts.tile([128, 128], F32)
            nc.sync.dma_start(tri_sb[:], tri_mask[:])
            eps_sb = consts.tile([1, 1], F32)
            nc.vector.memset(eps_sb[:], EPS)


            with tc.tile_pool(name="dramp", bufs=1, space="DRAM") as dp:
                g_in = dp.tile([HPC * D, T], F32)
                g_out = dp.tile([C, T], F32)

                with tc.tile_pool(name="qkvp", bufs=1) as qkvp:

                    with tc.tile_pool(name="xlnp", bufs=1) as xlnp:
                        xln = xlnp.tile([128, CC, T], F32)  # holds x, then ln(x) in place
                        xT = xln
                        for cc in range(CC):
                            nc.sync.dma_start(xT[:, cc, :], rearr_in(xT_d)[:, cc, :])

                        with tc.tile_pool(name="bc1p", bufs=1) as bc1p:
                            a_bc = bc1p.tile([128, T], F32)
                            c_bc = bc1p.tile([128, T], F32)

                            # ---- LN1 stats + broadcast ----
                            with tc.tile_pool(name="ln1p", bufs=1) as ln1p:
                                mu1 = ln1p.tile([1, T], F32)
                                msq1 = ln1p.tile([1, T], F32)
                                with tc.tile_pool(name="ln1_tmp", bufs=2) as tmp, \
                                     tc.tile_pool(name="ln1_ps", bufs=2, space="PSUM") as lps:
                                    for cch in range(T // 512):
                                        sl = slice(cch * 512, (cch + 1) * 512)
                                        ps_sum = lps.tile([1, 512], F32, tag="ps_sum")
                                        for cc in range(CC):
                                            nc.tensor.matmul(ps_sum[:], lhsT=inv768_sb[:],
                                                             rhs=xT[:, cc, sl],
                                                             start=(cc == 0), stop=(cc == CC - 1))
                                        nc.scalar.copy(mu1[:, sl], ps_sum[:])
                                        xsq = tmp.tile([128, CC, 512], F32, tag="xsq")
                                        for cc in range(CC):
                                            nc.scalar.square(xsq[:, cc, :], xT[:, cc, sl])
                                        ps_sq = lps.tile([1, 512], F32, tag="ps_sq")
                                        for cc in range(CC):
                                            nc.tensor.matmul(ps_sq[:], lhsT=inv768_sb[:],
                                                             rhs=xsq[:, cc, :],
                                                             start=(cc == 0), stop=(cc == CC - 1))
                                        nc.scalar.copy(msq1[:, sl], ps_sq[:])

                                rs1 = ln1p.tile([1, T], F32)
                                sm = msq1
                                cp1 = mu1
                                nc.vector.tensor_mul(rs1[:], mu1[:], mu1[:])
                                nc.vector.tensor_sub(sm[:], msq1[:], rs1[:])
                                nc.scalar.activation(sm[:], sm[:], AF.Sqrt, bias=eps_sb[:])
                                nc.vector.reciprocal(rs1[:], sm[:])
                                nc.vector.tensor_mul(cp1[:], mu1[:], rs1[:])

                                with tc.tile_pool(name="bc1_ps", bufs=2, space="PSUM") as bps:
                                    for cch in range(T // 512):
                                        sl = slice(cch * 512, (cch + 1) * 512)
                                        ps_a = bps.tile([128, 512], F32, tag="ps_a")
                                        nc.tensor.matmul(ps_a[:], lhsT=ones128_sb[:],
                                                         rhs=rs1[:, sl], start=True, stop=True)
                                        nc.scalar.copy(a_bc[:, sl], ps_a[:])
                                        ps_c = bps.tile([128, 512], F32, tag="ps_c")
                                        nc.tensor.matmul(ps_c[:], lhsT=negones128_sb[:],
                                                         rhs=cp1[:, sl], start=True, stop=True)
                                        nc.scalar.copy(c_bc[:, sl], ps_c[:])

                            # apply in place: x <- x*a_bc + c_bc
                            for cc in range(CC):
                                nc.vector.tensor_mul(xln[:, cc, :], xln[:, cc, :], a_bc[:])
                                nc.vector.tensor_add(xln[:, cc, :], xln[:, cc, :], c_bc[:])

                        if debug_taps:
                            for cc in range(CC):
                                nc.sync.dma_start(rearr_in(taps["xln"])[:, cc, :], xln[:, cc, :])

                        # ---- QKV ----
                        with tc.tile_pool(name="wqkv", bufs=1) as wp, \
                             tc.tile_pool(name="qkv_ps", bufs=3, space="PSUM") as qps:
                            qkT = qkvp.tile([128, CC, T], F32)   # q rows then k rows
                            v_aug = qkvp.tile([128, TT, HPC, D + 1], F32)
                            wqk_sb = wp.tile([128, CC, 2 * HPC * D], F32)
                            for cc in range(CC):
                                nc.sync.dma_start(wqk_sb[:, cc, :], rearr_in(wqk_d)[:, cc, :])
                            bqk_sb = wp.tile([128, CC], F32)
                            nc.sync.dma_start(bqk_sb[:], rearr_vec(bqk_d))
                            wv_sb = wp.tile([128, CC, HPC * D], F32)
                            for cc in range(CC):
                                nc.sync.dma_start(wv_sb[:, cc, :], rearr_in(wv_d)[:, cc, :])
                            bv_bc = wp.tile([128, HPC * D], F32)
                            nc.gpsimd.dma_start(bv_bc[:], bv_d[:].to_broadcast([128, HPC * D]))

                            for mc in range(CC):
                                for nch in range(T // 512):
                                    sl = slice(nch * 512, (nch + 1) * 512)
                                    ps = qps.tile([128, 512], F32, tag="ps_qk")
                                    for kc in range(CC):
                                        nc.tensor.matmul(
                                            ps[:], lhsT=wqk_sb[:, kc, mc * 128:(mc + 1) * 128],
                                            rhs=xln[:, kc, sl],
                                            start=(kc == 0), stop=(kc == CC - 1))
                                    nc.scalar.activation(qkT[:, mc, sl], ps[:], AF.Identity,
                                                         bias=bqk_sb[:, mc:mc + 1])

                            for tt in range(TT):
                                ps = qps.tile([128, HPC * D], F32, tag="ps_v")
                                for kc in range(CC):
                                    nc.tensor.matmul(
                                        ps[:], lhsT=xln[:, kc, tt * 128:(tt + 1) * 128],
                                        rhs=wv_sb[:, kc, :],
                                        start=(kc == 0), stop=(kc == CC - 1))
                                nc.vector.tensor_add(
                                    v_aug[:, tt, :, 0:D],
                                    ps[:].rearrange("p (h d) -> p h d", h=HPC),
                                    bv_bc[:].rearrange("p (h d) -> p h d", h=HPC))
                                nc.vector.memset(v_aug[:, tt, :, D:D + 1], 1.0)

                    if debug_taps:
                        for cc in range(CC):
                            nc.sync.dma_start(rearr_in(taps["qkT"])[:, cc, :], qkT[:, cc, :])
                        for tt in range(TT):
                            nc.sync.dma_start(
                                taps["v"][:].rearrange("(n p) m -> p n m", p=128)
                                [:, tt, :].rearrange("p (h d) -> p h d", h=HPC),
                                v_aug[:, tt, :, 0:D])

                    # ---- attention ----
                    with tc.tile_pool(name="ypool", bufs=1) as ypool:
                        yT = ypool.tile([128, HPC // 2, T], F32)
                        with tc.tile_pool(name="att_sb", bufs=4) as asb, \
                             tc.tile_pool(name="att_r", bufs=2) as arp, \
                             tc.tile_pool(name="att_ps_s", bufs=3, space="PSUM") as pss, \
                             tc.tile_pool(name="att_ps_y", bufs=2, space="PSUM") as psy, \
                             tc.tile_pool(name="att_ps_r", bufs=2, space="PSUM") as psr:
                            for h in range(HPC):
                                mq = h // 2
                                mk = HPC // 2 + h // 2
                                po = (h % 2) * 64
                                for qc in range(NQC):
                                    q0 = qc * QC
                                    nk = 4 * qc + 4
                                    ps_y = psy.tile([D + 1, QC], F32, tag="ps_y")
                                    for ki in range(nk):
                                        jj = ki - 4 * qc
                                        col_lo = jj * 128 if jj > 0 else 0
                                        csl = slice(col_lo, QC)
                                        ps_s = pss.tile([128, QC], F32, tag="ps_s")
                                        nc.tensor.matmul(
                                            ps_s[:, csl],
                                            lhsT=qkT[po:po + 64, mk, ki * 128:(ki + 1) * 128],
                                            rhs=qkT[po:po + 64, mq, q0 + col_lo:q0 + QC],
                                            start=True, stop=True)
                                        at = asb.tile([128, QC], F32, tag="attnT")
                                        nc.scalar.activation(at[:, csl], ps_s[:, csl], AF.Exp)
                                        if jj >= 0:
                                            nc.vector.tensor_mul(
                                                at[:, col_lo:col_lo + 128],
                                                at[:, col_lo:col_lo + 128], tri_sb[:])
                                        nc.tensor.matmul(
                                            ps_y[:, csl], lhsT=v_aug[:, ki, h, :],
                                            rhs=at[:, csl],
                                            start=(ki == 0), stop=(ki == nk - 1))
                                    r_sb = arp.tile([1, QC], F32, tag="r_sb")
                                    nc.vector.reciprocal(r_sb[:], ps_y[D:D + 1, :])
                                    ps_r = psr.tile([64, QC], F32, tag="ps_r")
                                    nc.tensor.matmul(ps_r[:], lhsT=ones64_sb[:], rhs=r_sb[:],
                                                     start=True, stop=True)
                                    r_bc = arp.tile([64, QC], F32, tag="r_bc")
                                    nc.scalar.copy(r_bc[:], ps_r[:])
                                    nc.vector.tensor_mul(yT[po:po + 64, mq, q0:q0 + QC],
                                                         ps_y[0:D, :], r_bc[:])

                        if debug_taps:
                            for cc in range(HPC // 2):
                                nc.sync.dma_start(rearr_in(taps["yT"])[:, cc, :], yT[:, cc, :])

                        for cc in range(HPC // 2):
                            nc.sync.dma_start(
                                g_in[:].rearrange("(c p) t -> p c t", p=128)[:, cc, :],
                                yT[:, cc, :])

                # ---- pair AllGather + proj ----
                nc.gpsimd.collective_compute(
                    "AllGather", mybir.AluOpType.bypass,
                    replica_groups=[[0, 1], [2, 3], [4, 5], [6, 7]],
                    ins=[g_in.opt()], outs=[g_out.opt()])

                pid = nc.sync.partition_id()
                toff = (pid % 2) * TH
                stack = contextlib.ExitStack()
                x2p = stack.enter_context(tc.tile_pool(name="x2p", bufs=1))
                x2 = x2p.tile([128, CC, TH], F32)
                with tc.tile_pool(name="ygp", bufs=1) as ygp:
                    yg = ygp.tile([128, CC, TH], F32)
                    for cc in range(CC):
                        nc.sync.dma_start(
                            yg[:, cc, :],
                            g_out[:].rearrange("(c p) t -> p c t", p=128)
                            [:, cc, bass.ds(toff, TH)])

                    with tc.tile_pool(name="wpp", bufs=1) as wpp, \
                         tc.tile_pool(name="proj_ps", bufs=3, space="PSUM") as pps:
                        wproj_sb = wpp.tile([128, CC, C], F32)
                        for cc in range(CC):
                            nc.sync.dma_start(wproj_sb[:, cc, :], rearr_in(wproj_d)[:, cc, :])
                        bproj_sb = wpp.tile([128, CC], F32)
                        nc.sync.dma_start(bproj_sb[:], rearr_vec(bproj_d))
                        xTh = wpp.tile([128, CC, TH], F32)
                        for cc in range(CC):
                            nc.sync.dma_start(xTh[:, cc, :], rearr_in(xTh_d)[:, cc, :])
                        for mc in range(CC):
                            for nch in range(TH // 512):
                                sl = slice(nch * 512, (nch + 1) * 512)
                                ps = pps.tile([128, 512], F32, tag="ps_proj")
                                for kc in range(CC):
                                    nc.tensor.matmul(
                                        ps[:], lhsT=wproj_sb[:, kc, mc * 128:(mc + 1) * 128],
                                        rhs=yg[:, kc, sl],
                                        start=(kc == 0), stop=(kc == CC - 1))
                                nc.vector.affine_then_add(
                                    x2[:, mc, sl], ps[:], xTh[:, mc, sl],
                                    scale=1.0, bias=bproj_sb[:, mc:mc + 1])

                if debug_taps:
                    for cc in range(CC):
                        nc.sync.dma_start(rearr_in(taps["x2"])[:, cc, :], x2[:, cc, :])

                # ---- LN2 + MLP ----
                with tc.tile_pool(name="xln2p", bufs=1) as xln2p:
                    xln2 = xln2p.tile([128, CC, TH], F32)
                    with tc.tile_pool(name="bc2p", bufs=1) as bc2p:
                        a_bc2 = bc2p.tile([128, TH], F32)
                        c_bc2 = bc2p.tile([128, TH], F32)
                        with tc.tile_pool(name="ln2p", bufs=1) as ln2p:
                            mu2 = ln2p.tile([1, TH], F32)
                            msq2 = ln2p.tile([1, TH], F32)
                            with tc.tile_pool(name="ln2_tmp", bufs=2) as tmp, \
                                 tc.tile_pool(name="ln2_ps", bufs=2, space="PSUM") as lps:
                                for cch in range(TH // 512):
                                    sl = slice(cch * 512, (cch + 1) * 512)
                                    ps_sum = lps.tile([1, 512], F32, tag="ps_sum2")
                                    for cc in range(CC):
                                        nc.tensor.matmul(ps_sum[:], lhsT=inv768_sb[:],
                                                         rhs=x2[:, cc, sl],
                                                         start=(cc == 0), stop=(cc == CC - 1))
                                    nc.scalar.copy(mu2[:, sl], ps_sum[:])
                                    xsq = tmp.tile([128, CC, 512], F32, tag="xsq2")
                                    for cc in range(CC):
                                        nc.scalar.square(xsq[:, cc, :], x2[:, cc, sl])
                                    ps_sq = lps.tile([1, 512], F32, tag="ps_sq2")
                                    for cc in range(CC):
                                        nc.tensor.matmul(ps_sq[:], lhsT=inv768_sb[:],
                                                         rhs=xsq[:, cc, :],
                                                         start=(cc == 0), stop=(cc == CC - 1))
                                    nc.scalar.copy(msq2[:, sl], ps_sq[:])

                            rs2 = ln2p.tile([1, TH], F32)
                            sm2 = msq2
                            cp2 = mu2
                            nc.vector.tensor_mul(rs2[:], mu2[:], mu2[:])
                            nc.vector.tensor_sub(sm2[:], msq2[:], rs2[:])
                            nc.scalar.activation(sm2[:], sm2[:], AF.Sqrt, bias=eps_sb[:])
                            nc.vector.reciprocal(rs2[:], sm2[:])
                            nc.vector.tensor_mul(cp2[:], mu2[:], rs2[:])

                            with tc.tile_pool(name="bc2_ps", bufs=2, space="PSUM") as bps:
                                for cch in range(TH // 512):
                                    sl = slice(cch * 512, (cch + 1) * 512)
                                    ps_a = bps.tile([128, 512], F32, tag="ps_a2")
                                    nc.tensor.matmul(ps_a[:], lhsT=ones128_sb[:], rhs=rs2[:, sl],
                                                     start=True, stop=True)
                                    nc.scalar.copy(a_bc2[:, sl], ps_a[:])
                                    ps_c = bps.tile([128, 512], F32, tag="ps_c2")
                                    nc.tensor.matmul(ps_c[:], lhsT=negones128_sb[:], rhs=cp2[:, sl],
                                                     start=True, stop=True)
                                    nc.scalar.copy(c_bc2[:, sl], ps_c[:])

                        for cc in range(CC):
                            nc.vector.tensor_mul(xln2[:, cc, :], x2[:, cc, :], a_bc2[:])
                            nc.vector.tensor_add(xln2[:, cc, :], xln2[:, cc, :], c_bc2[:])

                    if debug_taps:
                        for cc in range(CC):
                            nc.sync.dma_start(rearr_in(taps["xln2"])[:, cc, :], xln2[:, cc, :])

                    stack2 = contextlib.ExitStack()
                    hp = stack2.enter_context(tc.tile_pool(name="hp", bufs=1))
                    h_sb = hp.tile([128, FT, TH], BF16)
                    with tc.tile_pool(name="wfcp", bufs=1) as wfp, \
                         tc.tile_pool(name="fc_ps", bufs=3, space="PSUM") as fps:
                        wfc_sb = wfp.tile([128, CC, FF], F32)
                        for cc in range(CC):
                            nc.sync.dma_start(wfc_sb[:, cc, :], rearr_in(wfc_d)[:, cc, :])
                        bfc_sb = wfp.tile([128, FT], F32)
                        nc.sync.dma_start(bfc_sb[:], rearr_vec(bfc_d))
                        for mc in range(FT):
                            for nch in range(TH // 512):
                                sl = slice(nch * 512, (nch + 1) * 512)
                                ps = fps.tile([128, 512], F32, tag="ps_fc")
                                for kc in range(CC):
                                    nc.tensor.matmul(
                                        ps[:], lhsT=wfc_sb[:, kc, mc * 128:(mc + 1) * 128],
                                        rhs=xln2[:, kc, sl],
                                        start=(kc == 0), stop=(kc == CC - 1))
                                nc.scalar.activation(h_sb[:, mc, sl], ps[:],
                                                     AF.Gelu_apprx_tanh,
                                                     bias=bfc_sb[:, mc:mc + 1])

                    with tc.tile_pool(name="wfc2p", bufs=1) as w2p, \
                         tc.tile_pool(name="x3p", bufs=2) as x3p, \
                         tc.tile_pool(name="fc2_ps", bufs=3, space="PSUM") as f2ps:
                        wfc2_sb = w2p.tile([128, FT, C], BF16)
                        for ft in range(FT):
                            nc.sync.dma_start(wfc2_sb[:, ft, :], rearr_in(wfc2_d)[:, ft, :])
                        bfc2_sb = w2p.tile([128, CC], F32)
                        nc.sync.dma_start(bfc2_sb[:], rearr_vec(bfc2_d))
                        for mc in range(CC):
                            for nch in range(TH // 512):
                                sl = slice(nch * 512, (nch + 1) * 512)
                                ps = f2ps.tile([128, 512], F32, tag="ps_fc2")
                                for kc in range(FT):
                                    nc.tensor.matmul(
                                        ps[:], lhsT=wfc2_sb[:, kc, mc * 128:(mc + 1) * 128],
                                        rhs=h_sb[:, kc, sl],
                                        start=(kc == 0), stop=(kc == FT - 1))
                                x3 = x3p.tile([128, 512], F32, tag="x3")
                                nc.vector.affine_then_add(
                                    x3[:], ps[:], x2[:, mc, sl],
                                    scale=1.0, bias=bfc2_sb[:, mc:mc + 1])
                                nc.sync.dma_start(rearr_in(outT_d)[:, mc, sl], x3[:])

                    stack2.close()  # hp
                stack.close()  # x2p

    nc.finalize()
    return nc


_prog_cache = {}


def _get_program(debug_taps=False):
    key = debug_taps
    if key not in _prog_cache:
        _prog_cache[key] = build_program(debug_taps)
    return _prog_cache[key]


def make_in_maps(inputs):
    """Host-side sharding + weight prep. Returns per-core input dicts."""
    x = np.asarray(inputs["x"], np.float32)
    g1 = np.asarray(inputs["ln1_g"], np.float32)
    b1 = np.asarray(inputs["ln1_b"], np.float32)
    Wat = np.asarray(inputs["W_attn"], np.float32)
    bat = np.asarray(inputs["b_attn"], np.float32)
    Wp = np.asarray(inputs["W_proj"], np.float32)
    bp = np.asarray(inputs["b_proj"], np.float32)
    g2 = np.asarray(inputs["ln2_g"], np.float32)
    b2 = np.asarray(inputs["ln2_b"], np.float32)
    Wf = np.asarray(inputs["W_fc"], np.float32)
    bf = np.asarray(inputs["b_fc"], np.float32)
    Wf2 = np.asarray(inputs["W_fc2"], np.float32)
    bf2 = np.asarray(inputs["b_fc2"], np.float32)

    sc = 1.0 / math.sqrt(D)
    Wat_g = Wat * g1[:, None]              # fold LN gamma into W
    bat_eff = b1 @ Wat + bat               # fold LN beta into bias
    Wf_g = Wf * g2[:, None]
    bf_eff = b2 @ Wf + bf

    in_maps = []
    for c in range(NCORES):
        b, hf = c // 2, c % 2
        hsl = slice(hf * HPC * D, (hf + 1) * HPC * D)   # this core's head cols
        wq = Wat_g[:, 0:C][:, hsl] * sc
        bq = bat_eff[0:C][hsl] * sc
        wk = Wat_g[:, C:2 * C][:, hsl]
        bk = bat_eff[C:2 * C][hsl]
        wv = Wat_g[:, 2 * C:3 * C][:, hsl]
        bv = bat_eff[2 * C:3 * C][hsl]
        tsl = slice(hf * TH, (hf + 1) * TH)             # this core's token half
        in_maps.append({
            "xT": np.ascontiguousarray(x[b].T),
            "xTh": np.ascontiguousarray(x[b].T[:, tsl]),
            "wqk": np.ascontiguousarray(np.concatenate([wq, wk], axis=1)),
            "bqk": np.ascontiguousarray(np.concatenate([bq, bk])),
            "wv": np.ascontiguousarray(wv),
            "bv": np.ascontiguousarray(bv[None, :]),
            "wproj": np.ascontiguousarray(Wp),
            "bproj": bp,
            "wfc": np.ascontiguousarray(Wf_g),
            "bfc": bf_eff.astype(np.float32),
            "wfc2": Wf2.astype(ml_dtypes.bfloat16),
            "bfc2": bf2,
            "_tsl": tsl,
        })
    return in_maps


def kernel(**inputs):
    nc = _get_program(debug_taps=False)
    in_maps = make_in_maps(inputs)
    meta = [m.pop("_tsl") for m in in_maps]
    res = run_bass_kernel_spmd(nc, in_maps, list(range(NCORES)))
    out = np.empty((B, T, C), np.float32)
    for c in range(NCORES):
        b, tsl = c // 2, meta[c]
        out[b, tsl, :] = res.results[c]["outT"].T
    return out
